# revision 5
# baseline (speedup 1.0000x reference)
"""AdaptivePatchEmbed Trainium2 kernel.

Distribution: data-parallel over batch B=8 -> one sample per NeuronCore
(descriptors are identical across samples; small conv weights replicated).

Per-core device kernel:
  - scale-0 tokens: 88 static DRAM->DRAM block DMAs (one per 4x4 source
    block; dst rows are contiguous in desc0 order). Falls back to a
    dma_gather path if desc0 lacks the block structure.
  - scale-1 / scale-2 conv inputs: one dma_gather(transpose=True) each,
    which gathers the (i,j)-shifted rows for all tokens and deposits them
    channel-major as [128, D/128, ntok] bf16 -- directly the matmul rhs.
  - convs are einsums tok_out[d, n] = sum_{ij,c} W[(ij,c),d] * X[(ij,c),n],
    run as 128x128-chunk matmuls accumulating in PSUM (K-chunk order
    (i,j,c) matches the host-pretransposed weights).
  - conv2a output is written bf16 channel-major and feeds conv2b in-SBUF.
Outputs per core: out0 [1408,768] f32 (scale-0 copy) and outT [768,420]
f32 (tok1 ++ tok2 transposed). Host reassembles and computes positions.
"""

import numpy as np
import ml_dtypes
from contextlib import ExitStack

# Problem constants (hardcoded; kernel.py must be self-contained).
B, H, W, T, D = 8, 32, 32, 4, 768
N0, N1, N2 = 1408, 336, 84
NPOS = H * W * T              # 4096 rows of D
KC = 24                       # K chunks of 128 over (i,j,c)=4*768
MC = 6                        # output-d chunks of 128
NTOK1 = N1                    # 336 conv1 tokens
NTOK2A = N2 * 4               # 336 conv2a output tokens
N_CORES = 8

_compiled = {}


def _flat_idx(y, x, t):
    return (y * W + x) * T + t


def _wrap_idxs(idx, pad_to):
    """int sequence -> int16 SBUF wrap layout [128, pad_to//16]."""
    idx = np.asarray(idx, np.int64)
    full = np.zeros(pad_to, np.int64)  # pad with valid idx 0 (junk cols, never read)
    full[: idx.size] = idx
    assert full.max() < 32768 and pad_to % 16 == 0
    wrapped = full.reshape(pad_to // 16, 16).T.astype(np.int16)  # [16, cols]
    return np.tile(wrapped, (8, 1))  # replicate across the 8 groups of 16


def _weight_mat(w):
    """w [D, D, 2, 2] -> [(i,j,c)=4*D, D] bf16 so that
    wm[(i*2+j)*D + c, d] = w[d, c, i, j]."""
    wm = np.transpose(np.asarray(w, np.float32), (2, 3, 1, 0))  # [i, j, c, d]
    return np.ascontiguousarray(wm.reshape(4 * D, D)).astype(ml_dtypes.bfloat16)


def _bias_tile(b1, b2a, b2b):
    """[128, 18] f32: cols [g*6+m] = b_g[m*128+p]."""
    out = np.empty((128, 18), np.float32)
    for g, b in enumerate((b1, b2a, b2b)):
        out[:, g * 6 : (g + 1) * 6] = np.asarray(b, np.float32).reshape(MC, 128).T
    return out


def _tok0_blocks(desc0):
    """If desc0 is 88 4x4 raster blocks, return [(by, bx, t), ...] else None."""
    d0 = np.asarray(desc0, np.int64)
    if d0.shape != (N0, 3) or N0 % 16:
        return None
    blocks = d0.reshape(N0 // 16, 16, 3)
    by, bx, t = blocks[:, 0, 0], blocks[:, 0, 1], blocks[:, 0, 2]
    yy = by[:, None] + np.repeat(np.arange(4), 4)[None, :]
    xx = bx[:, None] + np.tile(np.arange(4), 4)[None, :]
    tt = np.broadcast_to(t[:, None], yy.shape)
    exp = np.stack([yy, xx, tt], axis=2)
    if not np.array_equal(exp, blocks) or yy.max() >= H or xx.max() >= W:
        return None
    return list(zip(by.tolist(), bx.tolist(), t.tolist()))


def _build_bass(tok0_blocks):
    import concourse.bacc as bacc
    import concourse.tile as tile
    from concourse import mybir

    nc = bacc.Bacc("TRN2", target_bir_lowering=False, debug=False,
                   num_devices=N_CORES, num_swdge_queues=1)
    dt = mybir.dt

    base_f32 = nc.dram_tensor("base_f32", (NPOS, D), dt.float32, kind="ExternalInput")
    base_bf16 = nc.dram_tensor("base_bf16", (NPOS, D), dt.bfloat16, kind="ExternalInput")
    w1m = nc.dram_tensor("w1m", (4 * D, D), dt.bfloat16, kind="ExternalInput")
    w2am = nc.dram_tensor("w2am", (4 * D, D), dt.bfloat16, kind="ExternalInput")
    w2bm = nc.dram_tensor("w2bm", (4 * D, D), dt.bfloat16, kind="ExternalInput")
    biases = nc.dram_tensor("biases", (128, 18), dt.float32, kind="ExternalInput")
    n_idx_cols = 176 if tok0_blocks is not None else 264
    idxs = nc.dram_tensor("idxs", (128, n_idx_cols), dt.int16, kind="ExternalInput")
    out0 = nc.dram_tensor("out0", (N0, D), dt.float32, kind="ExternalOutput")
    outT = nc.dram_tensor("outT", (D, NTOK1 + N2), dt.float32, kind="ExternalOutput")

    with ExitStack() as ctx:
        tc = ctx.enter_context(tile.TileContext(nc))
        consts = ctx.enter_context(tc.tile_pool(name="consts", bufs=1))
        wpool = ctx.enter_context(tc.tile_pool(name="wpool", bufs=1))
        gpool = ctx.enter_context(tc.tile_pool(name="gpool", bufs=1))
        opool = ctx.enter_context(tc.tile_pool(name="opool", bufs=1))
        psum = ctx.enter_context(tc.tile_pool(name="psum", bufs=4, space="PSUM"))

        # small consts on the scalar HWDGE ring so the sync ring starts w1 at t=0
        idx_s = consts.tile([128, n_idx_cols], dt.int16)
        nc.scalar.dma_start(idx_s[:], idxs.ap()[:])
        bias_s = consts.tile([128, 18], dt.float32)
        nc.scalar.dma_start(bias_s[:], biases.ap()[:])

        # conv gathers first: channel-major bf16 [128, 6, 1408]
        g1 = gpool.tile([128, MC, 1408], dt.bfloat16, tag="g1")
        nc.gpsimd.dma_gather(
            g1[:], base_bf16.ap()[:], idx_s[:, 0:88],
            num_idxs=1408, num_idxs_reg=1408, elem_size=D, transpose=True,
            single_packet=False,
        )
        g2a = gpool.tile([128, MC, 1408], dt.bfloat16, tag="g2a")
        nc.gpsimd.dma_gather(
            g2a[:], base_bf16.ap()[:], idx_s[:, 88:176],
            num_idxs=1408, num_idxs_reg=1408, elem_size=D, transpose=True,
            single_packet=False,
        )

        # Weights: [4D, D] viewed [KC, 128, D] -> SBUF [128, KC, D]
        wts = []
        for wdram, nm in ((w1m, "w1"), (w2am, "w2a"), (w2bm, "w2b")):
            wt = wpool.tile([128, KC, D], dt.bfloat16, tag=nm)
            src = wdram.ap().rearrange("(k p) d -> p k d", p=128)
            nc.sync.dma_start(wt[:], src)
            wts.append(wt)
        w1s, w2as, w2bs = wts

        # conv1: out1T[d, n] over 336 tokens
        out1 = opool.tile([128, MC, NTOK1], dt.float32, tag="out1")
        for m in range(MC):
            ps = psum.tile([128, NTOK1], dt.float32, tag="ps")
            for kc in range(KC):
                ij, c6 = divmod(kc, MC)
                nc.tensor.matmul(
                    ps[:],
                    w1s[:, kc, m * 128 : (m + 1) * 128],
                    g1[:, c6, ij * NTOK1 : (ij + 1) * NTOK1],
                    start=(kc == 0), stop=(kc == KC - 1),
                )
            nc.vector.tensor_scalar_add(out1[:, m, :], ps[:], bias_s[:, m : m + 1])

        # conv2a: 336 output tokens; token order inside each (i,j) group is
        # (h, w, n) so conv2b rhs slices are contiguous
        out2a = opool.tile([128, MC, NTOK2A], dt.bfloat16, tag="out2a")
        for m in range(MC):
            ps = psum.tile([128, NTOK2A], dt.float32, tag="ps")
            for kc in range(KC):
                ij, c6 = divmod(kc, MC)
                nc.tensor.matmul(
                    ps[:],
                    w2as[:, kc, m * 128 : (m + 1) * 128],
                    g2a[:, c6, ij * NTOK2A : (ij + 1) * NTOK2A],
                    start=(kc == 0), stop=(kc == KC - 1),
                )
            nc.vector.tensor_scalar_add(out2a[:, m, :], ps[:], bias_s[:, 6 + m : 7 + m])

        # conv2b: contracts conv2a output; group (i,j) -> cols [(2i+j)*84, +84)
        out2b = opool.tile([128, MC, N2], dt.float32, tag="out2b")
        for m in range(MC):
            ps = psum.tile([128, N2], dt.float32, tag="ps")
            for kc in range(KC):
                ij, c6 = divmod(kc, MC)
                nc.tensor.matmul(
                    ps[:],
                    w2bs[:, kc, m * 128 : (m + 1) * 128],
                    out2a[:, c6, ij * N2 : (ij + 1) * N2],
                    start=(kc == 0), stop=(kc == KC - 1),
                )
            nc.vector.tensor_scalar_add(out2b[:, m, :], ps[:], bias_s[:, 12 + m : 13 + m])

        # outputs: outT [768, 420] viewed [6, 128, 420]
        outT_v = outT.ap().rearrange("(m p) n -> p m n", p=128)
        nc.sync.dma_start(outT_v[:, :, 0:NTOK1], out1[:])
        nc.sync.dma_start(outT_v[:, :, NTOK1 : NTOK1 + N2], out2b[:])

        # scale-0: static DRAM->DRAM block copies (no Q7, no SBUF); emitted
        # last so they fill HWDGE/SDMA slack. Alternate both HWDGE rings.
        if tok0_blocks is not None:
            base_v = base_f32.ap().rearrange("(y x t) d -> y x t d", x=W, t=T)
            for k, (by, bx, t) in enumerate(tok0_blocks):
                eng = nc.scalar if (k % 2 == 0) else nc.sync
                eng.dma_start(
                    out0.ap()[k * 16 : (k + 1) * 16, :],
                    base_v[by : by + 4, bx : bx + 4, t, :],
                )
        else:
            g0 = gpool.tile([128, N0 // 128, D], dt.float32, tag="g0")
            nc.gpsimd.dma_gather(
                g0[:], base_f32.ap()[:], idx_s[:, 176:264],
                num_idxs=N0, num_idxs_reg=N0, elem_size=D, single_packet=False,
            )
            nc.sync.dma_start(
                out0.ap().rearrange("(g p) d -> p g d", p=128), g0[:]
            )

    nc.finalize()
    return nc


def _prep_shared(desc0, desc1, desc2, w1, b1, w2a, b2a, w2b, b2b):
    """Host-side shared (core-independent) input prep."""
    d0 = np.asarray(desc0, np.int64)
    d1 = np.asarray(desc1, np.int64)
    d2 = np.asarray(desc2, np.int64)

    tok0_blocks = _tok0_blocks(d0)

    # conv1: (i,j)-major groups, desc1 order inside
    idx1 = np.concatenate([
        _flat_idx(d1[:, 0] + i, d1[:, 1] + j, d1[:, 2])
        for i in range(2) for j in range(2)
    ])                                                                # [1344]

    # conv2a: (i,j)-major groups; token order inside = (h, w, n)
    hh, ww = np.arange(2), np.arange(2)
    idx2 = np.concatenate([
        _flat_idx(
            (d2[:, 0][None, None, :] + 2 * hh[:, None, None] + i),
            (d2[:, 1][None, None, :] + 2 * ww[None, :, None] + j),
            np.broadcast_to(d2[:, 2][None, None, :], (2, 2, N2)),
        ).ravel()
        for i in range(2) for j in range(2)
    ])                                                                # [1344]

    parts = [_wrap_idxs(idx1, 1408), _wrap_idxs(idx2, 1408)]
    if tok0_blocks is None:
        idx0 = _flat_idx(d0[:, 0], d0[:, 1], d0[:, 2])                # [1408]
        parts.append(_wrap_idxs(idx0, 1408))
    idxs = np.concatenate(parts, axis=1)

    shared = {
        "w1m": _weight_mat(w1),
        "w2am": _weight_mat(w2a),
        "w2bm": _weight_mat(w2b),
        "biases": _bias_tile(b1, b2a, b2b),
        "idxs": idxs,
    }

    def _pos(desc, size):
        n = desc.shape[0]
        return np.concatenate(
            [desc[:, :2].astype(np.int32),
             np.full((n, 1), size, np.int32),
             desc[:, 2:3].astype(np.int32)], axis=1)

    positions = np.concatenate(
        [_pos(np.asarray(desc0, np.int32), 1),
         _pos(np.asarray(desc1, np.int32), 2),
         _pos(np.asarray(desc2, np.int32), 4)], axis=0)
    return shared, positions, tok0_blocks


def kernel(base_patch_embeddings, desc0, desc1, desc2,
           w1, b1, w2a, b2a, w2b, b2b):
    from concourse.bass_utils import run_bass_kernel_spmd

    base = np.asarray(base_patch_embeddings, np.float32)
    assert base.shape == (B, H, W, T, D)

    shared, positions, tok0_blocks = _prep_shared(
        desc0, desc1, desc2, w1, b1, w2a, b2a, w2b, b2b)

    key = "blocks" if tok0_blocks is not None else "gather"
    if key not in _compiled:
        _compiled[key] = _build_bass(tok0_blocks)
    nc = _compiled[key]

    in_maps = []
    for b in range(B):
        sample = np.ascontiguousarray(base[b].reshape(NPOS, D))
        m = dict(shared)
        m["base_f32"] = sample
        m["base_bf16"] = sample.astype(ml_dtypes.bfloat16)
        in_maps.append(m)

    res = run_bass_kernel_spmd(nc, in_maps, core_ids=list(range(N_CORES)))

    tokens = np.empty((B, N0 + N1 + N2, D), np.float32)
    for b in range(B):
        tokens[b, :N0] = res.results[b]["out0"]
        tokens[b, N0:] = res.results[b]["outT"].T
    return tokens, positions


# revision 10
# speedup vs baseline: 1.0229x; 1.0229x over previous
"""AdaptivePatchEmbed Trainium2 kernel.

Distribution: data-parallel over batch B=8 -> one sample per NeuronCore
(descriptors are identical across samples; small conv weights replicated).

Per-core device kernel:
  - scale-0 tokens: 88 static DRAM->DRAM block DMAs (one per 4x4 source
    block; dst rows are contiguous in desc0 order). Falls back to a
    dma_gather path if desc0 lacks the block structure.
  - scale-1 / scale-2 conv inputs: one dma_gather(transpose=True) each,
    which gathers the (i,j)-shifted rows for all tokens and deposits them
    channel-major as [128, D/128, ntok] bf16 -- directly the matmul rhs.
  - convs are einsums tok_out[d, n] = sum_{ij,c} W[(ij,c),d] * X[(ij,c),n],
    run as 128x128-chunk matmuls accumulating in PSUM (K-chunk order
    (i,j,c) matches the host-pretransposed weights).
  - conv2a output is written bf16 channel-major and feeds conv2b in-SBUF.
Outputs per core: out0 [1408,768] f32 (scale-0 copy) and outT [768,420]
f32 (tok1 ++ tok2 transposed). Host reassembles and computes positions.
"""

import numpy as np
import ml_dtypes
from contextlib import ExitStack

# Problem constants (hardcoded; kernel.py must be self-contained).
B, H, W, T, D = 8, 32, 32, 4, 768
N0, N1, N2 = 1408, 336, 84
NPOS = H * W * T              # 4096 rows of D
KC = 24                       # K chunks of 128 over (i,j,c)=4*768
MC = 6                        # output-d chunks of 128
NTOK1 = N1                    # 336 conv1 tokens
NTOK2A = N2 * 4               # 336 conv2a output tokens
N_CORES = 8

_compiled = {}


def _flat_idx(y, x, t):
    return (y * W + x) * T + t


def _wrap_idxs(idx, pad_to):
    """int sequence -> int16 SBUF wrap layout [128, pad_to//16]."""
    idx = np.asarray(idx, np.int64)
    full = np.zeros(pad_to, np.int64)  # pad with valid idx 0 (junk cols, never read)
    full[: idx.size] = idx
    assert full.max() < 32768 and pad_to % 16 == 0
    wrapped = full.reshape(pad_to // 16, 16).T.astype(np.int16)  # [16, cols]
    return np.tile(wrapped, (8, 1))  # replicate across the 8 groups of 16


def _weight_mat(w):
    """w [D, D, 2, 2] -> [(i,j,c)=4*D, D] bf16 so that
    wm[(i*2+j)*D + c, d] = w[d, c, i, j]."""
    wm = np.transpose(np.asarray(w, np.float32), (2, 3, 1, 0))  # [i, j, c, d]
    return np.ascontiguousarray(wm.reshape(4 * D, D)).astype(ml_dtypes.bfloat16)


def _bias_tile(b1, b2a, b2b):
    """[128, 18] f32: cols [g*6+m] = b_g[m*128+p]."""
    out = np.empty((128, 18), np.float32)
    for g, b in enumerate((b1, b2a, b2b)):
        out[:, g * 6 : (g + 1) * 6] = np.asarray(b, np.float32).reshape(MC, 128).T
    return out


def _tok0_blocks(desc0):
    """If desc0 is 88 4x4 raster blocks, return [(by, bx, t), ...] else None."""
    d0 = np.asarray(desc0, np.int64)
    if d0.shape != (N0, 3) or N0 % 16:
        return None
    blocks = d0.reshape(N0 // 16, 16, 3)
    by, bx, t = blocks[:, 0, 0], blocks[:, 0, 1], blocks[:, 0, 2]
    yy = by[:, None] + np.repeat(np.arange(4), 4)[None, :]
    xx = bx[:, None] + np.tile(np.arange(4), 4)[None, :]
    tt = np.broadcast_to(t[:, None], yy.shape)
    exp = np.stack([yy, xx, tt], axis=2)
    if not np.array_equal(exp, blocks) or yy.max() >= H or xx.max() >= W:
        return None
    return list(zip(by.tolist(), bx.tolist(), t.tolist()))


def _build_bass(tok0_blocks):
    import concourse.bacc as bacc
    import concourse.tile as tile
    from concourse import mybir

    nc = bacc.Bacc("TRN2", target_bir_lowering=False, debug=False,
                   num_devices=N_CORES, num_swdge_queues=1,
                   dynamic_dma_scratch_size=32768)
    dt = mybir.dt

    base_f32 = nc.dram_tensor("base_f32", (NPOS, D), dt.float32, kind="ExternalInput")
    base_bf16 = nc.dram_tensor("base_bf16", (NPOS, D), dt.bfloat16, kind="ExternalInput")
    w1m = nc.dram_tensor("w1m", (4 * D, D), dt.bfloat16, kind="ExternalInput")
    w2am = nc.dram_tensor("w2am", (4 * D, D), dt.bfloat16, kind="ExternalInput")
    w2bm = nc.dram_tensor("w2bm", (4 * D, D), dt.bfloat16, kind="ExternalInput")
    biases = nc.dram_tensor("biases", (128, 18), dt.float32, kind="ExternalInput")
    n_idx_cols = 192 if tok0_blocks is not None else 280
    idxs = nc.dram_tensor("idxs", (128, n_idx_cols), dt.int16, kind="ExternalInput")
    out0 = nc.dram_tensor("out0", (N0, D), dt.float32, kind="ExternalOutput")
    outT = nc.dram_tensor("outT", (D, NTOK1 + N2), dt.float32, kind="ExternalOutput")

    with ExitStack() as ctx:
        tc = ctx.enter_context(tile.TileContext(nc))
        consts = ctx.enter_context(tc.tile_pool(name="consts", bufs=1))
        wpool = ctx.enter_context(tc.tile_pool(name="wpool", bufs=1))
        gpool = ctx.enter_context(tc.tile_pool(name="gpool", bufs=1))
        opool = ctx.enter_context(tc.tile_pool(name="opool", bufs=1))
        psum = ctx.enter_context(tc.tile_pool(name="psum", bufs=4, space="PSUM"))

        # idx upload first on the sync ring: it gates the gathers (critical path)
        idx_s = consts.tile([128, n_idx_cols], dt.int16)
        nc.sync.dma_start(idx_s[:], idxs.ap()[:])
        bias_s = consts.tile([128, 18], dt.float32)
        nc.scalar.dma_start(bias_s[:], biases.ap()[:])

        # conv gathers, split in halves (2 ij-groups each) so matmuls can
        # chase the gather data: channel-major bf16 [128, 6, 768] per half
        ghalves = []
        for gi in range(4):
            gh = gpool.tile([128, MC, 768], dt.bfloat16, tag=f"gh{gi}")
            nc.gpsimd.dma_gather(
                gh[:], base_bf16.ap()[:], idx_s[:, gi * 48 : gi * 48 + 48],
                num_idxs=768, num_idxs_reg=768, elem_size=D, transpose=True,
                single_packet=False,
            )
            ghalves.append(gh)

        def rhs_conv(gi_base, ij, c6, ntok):
            half = ghalves[gi_base + ij // 2]
            ijl = ij % 2
            return half[:, c6, ijl * ntok : (ijl + 1) * ntok]

        # Weights: [4D, D] viewed [KC, 128, D] -> SBUF [128, KC, D]
        wts = []
        for wdram, nm in ((w1m, "w1"), (w2am, "w2a"), (w2bm, "w2b")):
            wt = wpool.tile([128, KC, D], dt.bfloat16, tag=nm)
            src = wdram.ap().rearrange("(k p) d -> p k d", p=128)
            nc.sync.dma_start(wt[:], src)
            wts.append(wt)
        w1s, w2as, w2bs = wts

        # conv1: out1T[d, n] over 336 tokens
        out1 = opool.tile([128, MC, NTOK1], dt.float32, tag="out1")
        for m in range(MC):
            ps = psum.tile([128, NTOK1], dt.float32, tag="ps")
            for kc in range(KC):
                ij, c6 = divmod(kc, MC)
                nc.tensor.matmul(
                    ps[:],
                    w1s[:, kc, m * 128 : (m + 1) * 128],
                    rhs_conv(0, ij, c6, NTOK1),
                    start=(kc == 0), stop=(kc == KC - 1),
                )
            nc.vector.tensor_scalar_add(out1[:, m, :], ps[:], bias_s[:, m : m + 1])

        # conv2a: 336 output tokens; token order inside each (i,j) group is
        # (h, w, n) so conv2b rhs slices are contiguous
        out2a = opool.tile([128, MC, NTOK2A], dt.bfloat16, tag="out2a")
        for m in range(MC):
            ps = psum.tile([128, NTOK2A], dt.float32, tag="ps")
            for kc in range(KC):
                ij, c6 = divmod(kc, MC)
                nc.tensor.matmul(
                    ps[:],
                    w2as[:, kc, m * 128 : (m + 1) * 128],
                    rhs_conv(2, ij, c6, NTOK2A),
                    start=(kc == 0), stop=(kc == KC - 1),
                )
            nc.vector.tensor_scalar_add(out2a[:, m, :], ps[:], bias_s[:, 6 + m : 7 + m])

        # conv2b: contracts conv2a output; group (i,j) -> cols [(2i+j)*84, +84)
        out2b = opool.tile([128, MC, N2], dt.float32, tag="out2b")
        for m in range(MC):
            ps = psum.tile([128, N2], dt.float32, tag="ps")
            for kc in range(KC):
                ij, c6 = divmod(kc, MC)
                nc.tensor.matmul(
                    ps[:],
                    w2bs[:, kc, m * 128 : (m + 1) * 128],
                    out2a[:, c6, ij * N2 : (ij + 1) * N2],
                    start=(kc == 0), stop=(kc == KC - 1),
                )
            nc.vector.tensor_scalar_add(out2b[:, m, :], ps[:], bias_s[:, 12 + m : 13 + m])

        # scale-0: static DRAM->DRAM block copies (no Q7, no SBUF). Emitted
        # before the outT writes so they sit ahead of them in each HWDGE
        # ring's FIFO; split across both rings (scalar ring is mostly idle).
        if tok0_blocks is not None:
            base_v = base_f32.ap().rearrange("(y x t) d -> y x t d", x=W, t=T)
            for k, (by, bx, t) in enumerate(tok0_blocks):
                eng = nc.scalar if (k % 2 == 0) else nc.sync
                eng.dma_start(
                    out0.ap()[k * 16 : (k + 1) * 16, :],
                    base_v[by : by + 4, bx : bx + 4, t, :],
                )
        else:
            g0 = gpool.tile([128, N0 // 128, D], dt.float32, tag="g0")
            nc.gpsimd.dma_gather(
                g0[:], base_f32.ap()[:], idx_s[:, 192:280],
                num_idxs=N0, num_idxs_reg=N0, elem_size=D, single_packet=False,
            )
            nc.sync.dma_start(
                out0.ap().rearrange("(g p) d -> p g d", p=128), g0[:]
            )

        # outputs: outT [768, 420] viewed [6, 128, 420]
        outT_v = outT.ap().rearrange("(m p) n -> p m n", p=128)
        nc.sync.dma_start(outT_v[:, :, 0:NTOK1], out1[:])
        nc.sync.dma_start(outT_v[:, :, NTOK1 : NTOK1 + N2], out2b[:])

    nc.finalize()
    return nc


def _prep_shared(desc0, desc1, desc2, w1, b1, w2a, b2a, w2b, b2b):
    """Host-side shared (core-independent) input prep."""
    d0 = np.asarray(desc0, np.int64)
    d1 = np.asarray(desc1, np.int64)
    d2 = np.asarray(desc2, np.int64)

    tok0_blocks = _tok0_blocks(d0)

    # conv1: (i,j)-major groups, desc1 order inside
    idx1_groups = [
        _flat_idx(d1[:, 0] + i, d1[:, 1] + j, d1[:, 2])
        for i in range(2) for j in range(2)
    ]                                                                 # 4 x [336]

    # conv2a: (i,j)-major groups; token order inside = (h, w, n)
    hh, ww = np.arange(2), np.arange(2)
    idx2_groups = [
        _flat_idx(
            (d2[:, 0][None, None, :] + 2 * hh[:, None, None] + i),
            (d2[:, 1][None, None, :] + 2 * ww[None, :, None] + j),
            np.broadcast_to(d2[:, 2][None, None, :], (2, 2, N2)),
        ).ravel()
        for i in range(2) for j in range(2)
    ]                                                                 # 4 x [336]

    # 4 gather halves of 768 idxs (2 ij-groups + pad) matching _build_bass
    halves = [np.concatenate(idx1_groups[0:2]), np.concatenate(idx1_groups[2:4]),
              np.concatenate(idx2_groups[0:2]), np.concatenate(idx2_groups[2:4])]
    parts = [_wrap_idxs(h, 768) for h in halves]
    if tok0_blocks is None:
        idx0 = _flat_idx(d0[:, 0], d0[:, 1], d0[:, 2])                # [1408]
        parts.append(_wrap_idxs(idx0, 1408))
    idxs = np.concatenate(parts, axis=1)

    shared = {
        "w1m": _weight_mat(w1),
        "w2am": _weight_mat(w2a),
        "w2bm": _weight_mat(w2b),
        "biases": _bias_tile(b1, b2a, b2b),
        "idxs": idxs,
    }

    def _pos(desc, size):
        n = desc.shape[0]
        return np.concatenate(
            [desc[:, :2].astype(np.int32),
             np.full((n, 1), size, np.int32),
             desc[:, 2:3].astype(np.int32)], axis=1)

    positions = np.concatenate(
        [_pos(np.asarray(desc0, np.int32), 1),
         _pos(np.asarray(desc1, np.int32), 2),
         _pos(np.asarray(desc2, np.int32), 4)], axis=0)
    return shared, positions, tok0_blocks


def kernel(base_patch_embeddings, desc0, desc1, desc2,
           w1, b1, w2a, b2a, w2b, b2b):
    from concourse.bass_utils import run_bass_kernel_spmd

    base = np.asarray(base_patch_embeddings, np.float32)
    assert base.shape == (B, H, W, T, D)

    shared, positions, tok0_blocks = _prep_shared(
        desc0, desc1, desc2, w1, b1, w2a, b2a, w2b, b2b)

    key = "blocks" if tok0_blocks is not None else "gather"
    if key not in _compiled:
        _compiled[key] = _build_bass(tok0_blocks)
    nc = _compiled[key]

    in_maps = []
    for b in range(B):
        sample = np.ascontiguousarray(base[b].reshape(NPOS, D))
        m = dict(shared)
        m["base_f32"] = sample
        m["base_bf16"] = sample.astype(ml_dtypes.bfloat16)
        in_maps.append(m)

    res = run_bass_kernel_spmd(nc, in_maps, core_ids=list(range(N_CORES)))

    tokens = np.empty((B, N0 + N1 + N2, D), np.float32)
    for b in range(B):
        tokens[b, :N0] = res.results[b]["out0"]
        tokens[b, N0:] = res.results[b]["outT"].T
    return tokens, positions


# revision 21
# speedup vs baseline: 1.0571x; 1.0334x over previous
"""AdaptivePatchEmbed Trainium2 kernel.

Distribution: data-parallel over batch B=8 -> one sample per NeuronCore
(descriptors are identical across samples; small conv weights replicated).

Per-core device kernel:
  - scale-0 tokens: 88 static DRAM->DRAM block DMAs (one per 4x4 source
    block; dst rows are contiguous in desc0 order). Falls back to a
    dma_gather path if desc0 lacks the block structure.
  - scale-1 / scale-2 conv inputs: one dma_gather(transpose=True) each,
    which gathers the (i,j)-shifted rows for all tokens and deposits them
    channel-major as [128, D/128, ntok] bf16 -- directly the matmul rhs.
  - convs are einsums tok_out[d, n] = sum_{ij,c} W[(ij,c),d] * X[(ij,c),n],
    run as 128x128-chunk matmuls accumulating in PSUM (K-chunk order
    (i,j,c) matches the host-pretransposed weights).
  - conv2a output is written bf16 channel-major and feeds conv2b in-SBUF.
Outputs per core: out0 [1408,768] f32 (scale-0 copy) and outT [768,420]
f32 (tok1 ++ tok2 transposed). Host reassembles and computes positions.
"""

import numpy as np
import ml_dtypes
from contextlib import ExitStack

# Problem constants (hardcoded; kernel.py must be self-contained).
B, H, W, T, D = 8, 32, 32, 4, 768
N0, N1, N2 = 1408, 336, 84
NPOS = H * W * T              # 4096 rows of D
KC = 24                       # K chunks of 128 over (i,j,c)=4*768
MC = 6                        # output-d chunks of 128
NTOK1 = N1                    # 336 conv1 tokens
NTOK2A = N2 * 4               # 336 conv2a output tokens
N_CORES = 8

_compiled = {}


def _flat_idx(y, x, t):
    return (y * W + x) * T + t


def _wrap_idxs(idx, pad_to):
    """int sequence -> int16 SBUF wrap layout [128, pad_to//16]."""
    idx = np.asarray(idx, np.int64)
    full = np.zeros(pad_to, np.int64)  # pad with valid idx 0 (junk cols, never read)
    full[: idx.size] = idx
    assert full.max() < 32768 and pad_to % 16 == 0
    wrapped = full.reshape(pad_to // 16, 16).T.astype(np.int16)  # [16, cols]
    return np.tile(wrapped, (8, 1))  # replicate across the 8 groups of 16


def _weight_mat(w):
    """w [D, D, 2, 2] -> [(i,j,c)=4*D, D] bf16 so that
    wm[(i*2+j)*D + c, d] = w[d, c, i, j]."""
    wm = np.transpose(np.asarray(w, np.float32), (2, 3, 1, 0))  # [i, j, c, d]
    return np.ascontiguousarray(wm.reshape(4 * D, D)).astype(ml_dtypes.bfloat16)


def _bias_tile(b1, b2a, b2b):
    """[128, 18] f32: cols [g*6+m] = b_g[m*128+p]."""
    out = np.empty((128, 18), np.float32)
    for g, b in enumerate((b1, b2a, b2b)):
        out[:, g * 6 : (g + 1) * 6] = np.asarray(b, np.float32).reshape(MC, 128).T
    return out


def _tok0_blocks(desc0):
    """If desc0 is 88 4x4 raster blocks, return [(by, bx, t), ...] else None."""
    d0 = np.asarray(desc0, np.int64)
    if d0.shape != (N0, 3) or N0 % 16:
        return None
    blocks = d0.reshape(N0 // 16, 16, 3)
    by, bx, t = blocks[:, 0, 0], blocks[:, 0, 1], blocks[:, 0, 2]
    yy = by[:, None] + np.repeat(np.arange(4), 4)[None, :]
    xx = bx[:, None] + np.tile(np.arange(4), 4)[None, :]
    tt = np.broadcast_to(t[:, None], yy.shape)
    exp = np.stack([yy, xx, tt], axis=2)
    if not np.array_equal(exp, blocks) or yy.max() >= H or xx.max() >= W:
        return None
    return list(zip(by.tolist(), bx.tolist(), t.tolist()))


def _tok0_runs(tok0_blocks):
    """Batch the scale-0 copy into few fat DMAs.

    Requires the spatial 4x4 block set to be identical across all T
    timesteps and 4-aligned. Returns (runs, perm):
      runs: [(by, bx_blk0, step_blk, L)] -- each run is one DRAM->DRAM DMA
            covering [4 y, L blocks strided, 16 rows (4x * 4t)] of base.
      perm: int array s.t. out0_final = out0_raw[perm] (host-side reorder).
    """
    if tok0_blocks is None:
        return None, None
    byt = {}
    for by, bx, t in tok0_blocks:
        if by % 4 or bx % 4:
            return None, None
        byt.setdefault((by, bx), set()).add(t)
    if any(ts != set(range(T)) for ts in byt.values()):
        return None, None
    if len(byt) * 16 * T != N0:
        return None, None
    rows = {}
    for (by, bx) in byt:
        rows.setdefault(by, []).append(bx // 4)
    runs = []
    for by in sorted(rows):
        bxs = sorted(rows[by])
        i = 0
        while i < len(bxs):
            j = i + 1
            step = 1
            if j < len(bxs):
                step = bxs[j] - bxs[i]
                while j + 1 < len(bxs) and bxs[j + 1] - bxs[j] == step:
                    j += 1
            runs.append((by, bxs[i], step, j - i))
            i = j
    # raw row order produced by the DMAs
    raw = []
    for by, b0, s, L in runs:
        for dy in range(4):
            for k in range(L):
                for dx in range(4):
                    for t in range(T):
                        raw.append(((by + dy) * W + (b0 + k * s) * 4 + dx) * T + t)
    raw = np.asarray(raw)
    pos_of = np.full(NPOS, -1, np.int64)
    pos_of[raw] = np.arange(N0)
    return runs, pos_of


def _build_bass(tok0_runs):
    import concourse.bacc as bacc
    import concourse.tile as tile
    from concourse import mybir

    nc = bacc.Bacc("TRN2", target_bir_lowering=False, debug=False,
                   num_devices=N_CORES, num_swdge_queues=1,
                   dynamic_dma_scratch_size=32768)
    dt = mybir.dt

    base_f32 = nc.dram_tensor("base_f32", (NPOS, D), dt.float32, kind="ExternalInput")
    base_bf16 = nc.dram_tensor("base_bf16", (NPOS, D), dt.bfloat16, kind="ExternalInput")
    w1m = nc.dram_tensor("w1m", (4 * D, D), dt.bfloat16, kind="ExternalInput")
    w2am = nc.dram_tensor("w2am", (4 * D, D), dt.bfloat16, kind="ExternalInput")
    w2bm = nc.dram_tensor("w2bm", (4 * D, D), dt.bfloat16, kind="ExternalInput")
    biases = nc.dram_tensor("biases", (128, 18), dt.float32, kind="ExternalInput")
    n_idx_cols = 192 if tok0_runs is not None else 280
    idxs = nc.dram_tensor("idxs", (128, n_idx_cols), dt.int16, kind="ExternalInput")
    out0 = nc.dram_tensor("out0", (N0, D), dt.float32, kind="ExternalOutput")
    outT = nc.dram_tensor("outT", (D, NTOK1 + N2), dt.float32, kind="ExternalOutput")

    with ExitStack() as ctx:
        tc = ctx.enter_context(tile.TileContext(nc))
        consts = ctx.enter_context(tc.tile_pool(name="consts", bufs=1))
        wpool = ctx.enter_context(tc.tile_pool(name="wpool", bufs=1))
        gpool = ctx.enter_context(tc.tile_pool(name="gpool", bufs=1))
        opool = ctx.enter_context(tc.tile_pool(name="opool", bufs=1))
        psum = ctx.enter_context(tc.tile_pool(name="psum", bufs=4, space="PSUM"))

        # Dummy tiny gather first: forces the gpsimd dma_gather ucode library
        # load to queue before the big weight DMAs (else the real gathers
        # stall ~15us waiting for the library DMA stuck behind w1).
        idx_d = consts.tile([128, 8], dt.int16)
        nc.gpsimd.memset(idx_d[:], 0)
        scrap = consts.tile([128, 1, 128], dt.bfloat16)
        nc.gpsimd.dma_gather(
            scrap[:], base_bf16.ap().rearrange("n (a b) -> (n a) b", b=128),
            idx_d[:], num_idxs=128, num_idxs_reg=128, elem_size=128,
            single_packet=False,
        )

        # idx upload first on the sync ring: it gates the gathers (critical path)
        idx_s = consts.tile([128, n_idx_cols], dt.int16)
        nc.sync.dma_start(idx_s[:], idxs.ap()[:])
        bias_s = consts.tile([128, 18], dt.float32)
        nc.scalar.dma_start(bias_s[:], biases.ap()[:])

        # conv gathers, split in halves (2 ij-groups each) so matmuls can
        # chase the gather data: channel-major bf16 [128, 6, 768] per half
        ghalves = []
        for gi in range(4):
            gh = gpool.tile([128, MC, 768], dt.bfloat16, tag=f"gh{gi}")
            nc.gpsimd.dma_gather(
                gh[:], base_bf16.ap()[:], idx_s[:, gi * 48 : gi * 48 + 48],
                num_idxs=768, num_idxs_reg=768, elem_size=D, transpose=True,
                single_packet=False,
            )
            ghalves.append(gh)

        def rhs_conv(gi_base, ij, c6, ntok):
            half = ghalves[gi_base + ij // 2]
            ijl = ij % 2
            return half[:, c6, ijl * ntok : (ijl + 1) * ntok]

        # Weights: [4D, D] viewed [KC, 128, D] -> SBUF [128, KC, D]
        wts = []
        for wdram, nm in ((w1m, "w1"), (w2am, "w2a"), (w2bm, "w2b")):
            wt = wpool.tile([128, KC, D], dt.bfloat16, tag=nm)
            src = wdram.ap().rearrange("(k p) d -> p k d", p=128)
            nc.sync.dma_start(wt[:], src)
            wts.append(wt)
        w1s, w2as, w2bs = wts

        # conv1: out1T[d, n] over 336 tokens
        out1 = opool.tile([128, MC, NTOK1], dt.float32, tag="out1")
        for m in range(MC):
            ps = psum.tile([128, NTOK1], dt.float32, tag="ps")
            for kc in range(KC):
                ij, c6 = divmod(kc, MC)
                nc.tensor.matmul(
                    ps[:],
                    w1s[:, kc, m * 128 : (m + 1) * 128],
                    rhs_conv(0, ij, c6, NTOK1),
                    start=(kc == 0), stop=(kc == KC - 1),
                )
            nc.vector.tensor_scalar_add(out1[:, m, :], ps[:], bias_s[:, m : m + 1])

        # conv2a: 336 output tokens; token order inside each (i,j) group is
        # (h, w, n) so conv2b rhs slices are contiguous
        out2a = opool.tile([128, MC, NTOK2A], dt.bfloat16, tag="out2a")
        for m in range(MC):
            ps = psum.tile([128, NTOK2A], dt.float32, tag="ps")
            for kc in range(KC):
                ij, c6 = divmod(kc, MC)
                nc.tensor.matmul(
                    ps[:],
                    w2as[:, kc, m * 128 : (m + 1) * 128],
                    rhs_conv(2, ij, c6, NTOK2A),
                    start=(kc == 0), stop=(kc == KC - 1),
                )
            nc.vector.tensor_scalar_add(out2a[:, m, :], ps[:], bias_s[:, 6 + m : 7 + m])

        # conv2b: contracts conv2a output; group (i,j) -> cols [(2i+j)*84, +84)
        out2b = opool.tile([128, MC, N2], dt.float32, tag="out2b")
        for m in range(MC):
            ps = psum.tile([128, N2], dt.float32, tag="ps")
            for kc in range(KC):
                ij, c6 = divmod(kc, MC)
                nc.tensor.matmul(
                    ps[:],
                    w2bs[:, kc, m * 128 : (m + 1) * 128],
                    out2a[:, c6, ij * N2 : (ij + 1) * N2],
                    start=(kc == 0), stop=(kc == KC - 1),
                )
            nc.vector.tensor_scalar_add(out2b[:, m, :], ps[:], bias_s[:, 12 + m : 13 + m])

        # scale-0: few fat static DRAM->DRAM copies in SOURCE order (the
        # host applies the row permutation when assembling the output).
        # On the scalar ring, which is otherwise idle.
        if tok0_runs is not None:
            base_y = base_f32.ap().rearrange("(y bx rr) d -> y bx rr d",
                                             bx=W // 4, rr=4 * T)
            off = 0
            for by, b0, s, L in tok0_runs:
                nrows = 4 * L * 4 * T
                nc.scalar.dma_start(
                    out0.ap()[off : off + nrows, :],
                    base_y[by : by + 4, b0 : b0 + (L - 1) * s + 1 : s, :, :],
                )
                off += nrows
        else:
            g0 = gpool.tile([128, N0 // 128, D], dt.float32, tag="g0")
            nc.gpsimd.dma_gather(
                g0[:], base_f32.ap()[:], idx_s[:, 192:280],
                num_idxs=N0, num_idxs_reg=N0, elem_size=D, single_packet=False,
            )
            nc.sync.dma_start(
                out0.ap().rearrange("(g p) d -> p g d", p=128), g0[:]
            )

        # outputs: outT [768, 420] viewed [6, 128, 420]
        outT_v = outT.ap().rearrange("(m p) n -> p m n", p=128)
        nc.sync.dma_start(outT_v[:, :, 0:NTOK1], out1[:])
        nc.sync.dma_start(outT_v[:, :, NTOK1 : NTOK1 + N2], out2b[:])

    nc.finalize()
    return nc


def _prep_shared(desc0, desc1, desc2, w1, b1, w2a, b2a, w2b, b2b):
    """Host-side shared (core-independent) input prep."""
    d0 = np.asarray(desc0, np.int64)
    d1 = np.asarray(desc1, np.int64)
    d2 = np.asarray(desc2, np.int64)

    tok0_blocks = _tok0_blocks(d0)
    tok0_runs, pos_of = _tok0_runs(tok0_blocks)
    if tok0_runs is not None:
        d0flat = _flat_idx(d0[:, 0], d0[:, 1], d0[:, 2])
        perm = pos_of[d0flat]
        assert perm.min() >= 0
    else:
        perm = None

    # conv1: (i,j)-major groups, desc1 order inside
    idx1_groups = [
        _flat_idx(d1[:, 0] + i, d1[:, 1] + j, d1[:, 2])
        for i in range(2) for j in range(2)
    ]                                                                 # 4 x [336]

    # conv2a: (i,j)-major groups; token order inside = (h, w, n)
    hh, ww = np.arange(2), np.arange(2)
    idx2_groups = [
        _flat_idx(
            (d2[:, 0][None, None, :] + 2 * hh[:, None, None] + i),
            (d2[:, 1][None, None, :] + 2 * ww[None, :, None] + j),
            np.broadcast_to(d2[:, 2][None, None, :], (2, 2, N2)),
        ).ravel()
        for i in range(2) for j in range(2)
    ]                                                                 # 4 x [336]

    # 4 gather halves of 768 idxs (2 ij-groups + pad) matching _build_bass
    halves = [np.concatenate(idx1_groups[0:2]), np.concatenate(idx1_groups[2:4]),
              np.concatenate(idx2_groups[0:2]), np.concatenate(idx2_groups[2:4])]
    parts = [_wrap_idxs(h, 768) for h in halves]
    if tok0_runs is None:
        idx0 = _flat_idx(d0[:, 0], d0[:, 1], d0[:, 2])                # [1408]
        parts.append(_wrap_idxs(idx0, 1408))
    idxs = np.concatenate(parts, axis=1)

    shared = {
        "w1m": _weight_mat(w1),
        "w2am": _weight_mat(w2a),
        "w2bm": _weight_mat(w2b),
        "biases": _bias_tile(b1, b2a, b2b),
        "idxs": idxs,
    }

    def _pos(desc, size):
        n = desc.shape[0]
        return np.concatenate(
            [desc[:, :2].astype(np.int32),
             np.full((n, 1), size, np.int32),
             desc[:, 2:3].astype(np.int32)], axis=1)

    positions = np.concatenate(
        [_pos(np.asarray(desc0, np.int32), 1),
         _pos(np.asarray(desc1, np.int32), 2),
         _pos(np.asarray(desc2, np.int32), 4)], axis=0)
    return shared, positions, tok0_runs, perm


def kernel(base_patch_embeddings, desc0, desc1, desc2,
           w1, b1, w2a, b2a, w2b, b2b):
    from concourse.bass_utils import run_bass_kernel_spmd

    base = np.asarray(base_patch_embeddings, np.float32)
    assert base.shape == (B, H, W, T, D)

    shared, positions, tok0_runs, perm = _prep_shared(
        desc0, desc1, desc2, w1, b1, w2a, b2a, w2b, b2b)

    key = repr(tok0_runs)
    if key not in _compiled:
        _compiled[key] = _build_bass(tok0_runs)
    nc = _compiled[key]

    in_maps = []
    for b in range(B):
        sample = np.ascontiguousarray(base[b].reshape(NPOS, D))
        m = dict(shared)
        m["base_f32"] = sample
        m["base_bf16"] = sample.astype(ml_dtypes.bfloat16)
        in_maps.append(m)

    res = run_bass_kernel_spmd(nc, in_maps, core_ids=list(range(N_CORES)))

    tokens = np.empty((B, N0 + N1 + N2, D), np.float32)
    for b in range(B):
        out0 = res.results[b]["out0"]
        tokens[b, :N0] = out0[perm] if perm is not None else out0
        tokens[b, N0:] = res.results[b]["outT"].T
    return tokens, positions
